# revision 5
# baseline (speedup 1.0000x reference)
"""Trainium2 Bass kernel for the e3nn-style point kernel:

    out[z, i, j] = sum_{y,w} Q[i,j,y,w] * Ysh[z,y] * Rad[z,w]      (+ K0 fallback
                                                                     for |r|==0)
    Ysh = real spherical harmonics l=0,1,2 of d = r/|r|  (component norm)
    Rad = relu(|r| * W1 + b1) @ W2 + b2

Strategy (per core, data-parallel over N across 8 cores, NP=8192 points each):
  Feature-major construction of the Khatri-Rao features F^T[(y,w), z], and
  (critically for this execution environment) a MINIMAL number of DMA
  instructions -- measured HW cost is ~2ms per DMA instruction regardless of
  size, so all constants + the per-core r shard travel in ONE host-packed
  blob DMA and the whole output leaves in ONE DMA (fp16, cast on host).

  - Point-major radii/Ysh on DVE; radii go feature-major via one PE
    transpose; Ysh rows via PE transposes ([128,128] tiles, Ysh padded to
    32 y-slots so transposed rows land 32-partition-aligned).
  - hidden h^T[h, z] via 64 K=64 PE matmuls with host-packed selector
    stationaries (w1rep[q] = e_q outer W1) against the transposed radii
    tile (avoids a flatten DMA), + ACT relu.
  - RadT[w, z] via PE (lhsT=W2) directly feature-major.
  - Y broadcast rows Ybc_kt[p, z] = Y[z, y(128kt+p)] via tiny K=9 PE
    matmuls with 0/1 selector matrices (4 concurrent row-groups).
  - Rad replicas rad9[p, slot, z] = Rad[z, w(128kt+p)] via 32-row
    quadrant-aligned DVE copies (cross-partition moves are only legal
    quadrant-aligned <=32 rows; shift-free segments any size).
  - F^T = Ybc * rad9 elementwise on DVE (split: some k-tiles via ACT
    PSUM->SBUF copy then 2x fp16 DVE mult, rest direct from PSUM).
  - Main GEMM: out[z, ij] = sum_k F^T[k, z] * Qstack[k, ij], 7 K-tiles
    PSUM-accumulated, Qstack = [Qmat; B; 0] host-prepacked fp16.
  Point order inside a core is block-permuted (z = 64*p + j); the output
  DMA un-permutes for free (each partition's 64 rows are contiguous).
"""

import math
from contextlib import ExitStack

import numpy as np

import concourse.bass as bass
import concourse.mybir as mybir
import concourse.tile as tile
from concourse import bacc
from concourse._compat import with_exitstack
from concourse.bass import ds, ts
from concourse.bass_utils import run_bass_kernel_spmd
from concourse.masks import make_identity

F32 = mybir.dt.float32
F16 = mybir.dt.float16
AF = mybir.ActivationFunctionType
OP = mybir.AluOpType

N_TOTAL = 65536
N_CORES = 8
NP = N_TOTAL // N_CORES          # 8192 points per core
NT = NP // 128                   # 64 z-tiles of 128 points
NCH = NT // 4                    # 16 chunks of 4 z-tiles (512 points)
H = 128                          # MLP hidden
W = 96                           # MLP out / radial channels
NY = 9                           # spherical harmonics
NYP = 32                         # padded y-slots (transpose alignment)
IJ = 256                         # 16*16 outputs
KF = 9 * W                       # 864 true feature rows
KV = KF + NY                     # 873 rows incl. Ysh block (for the B term)
KT = 7                           # K-tiles of 128 (896 rows, 873 valid)

SQ3 = math.sqrt(3.0)
SQ5 = math.sqrt(5.0)
SQ15 = math.sqrt(15.0)

# fp16-element offsets of the per-partition regions inside the input blob
QOFF = 0                         # qstack   [7, 256]
MOFF = QOFF + KT * IJ            # mks      [7, 128]
W2OFF = MOFF + KT * 128          # w2       [96]
W1ROFF = W2OFF + W               # w1rep    [64, 128]
B1OFF = W1ROFF + NT * H          # b1       f32 [1]  (2 fp16 slots)
ROFF = B1OFF + 2                 # r        f32 [64, 3] (384 fp16 slots)
BLOB = ROFF + NT * 3 * 2         # total fp16 elems per partition

# rad9 slot layout: slot s holds rows p -> radt[w] per the (y,w) k-tile maps.
# k-tiles 0..5 repeat with period 3 (128*3 == 96*4); k-tile 6 is slot 3.
RAD9_SLOT = [0, 1, 2, 0, 1, 2, 3]
# (slot, out_base, in_base, nrows): partition-moving copies decomposed into
# 32-row quadrant-aligned pieces (DVE cross-partition constraint); the
# shift-free 96-row segments go as single ops.
RAD9_COPIES = [
    (0, 0, 0, 96),
    (0, 96, 0, 32),
    (1, 0, 32, 32), (1, 32, 64, 32),
    (1, 64, 0, 32), (1, 96, 32, 32),
    (2, 0, 64, 32),
    (2, 32, 0, 32), (2, 64, 32, 32), (2, 96, 64, 32),
    (3, 0, 0, 96),
]
# k-tiles whose F-mult goes via ACT copy to SBUF + 2x fp16 DVE mult
ACT_KTS = (0, 1, 2)


@with_exitstack
def _emit(ctx: ExitStack, tc: tile.TileContext, blob_ext, out_ext):
    nc = tc.nc

    consts = ctx.enter_context(tc.tile_pool(name="consts", bufs=1))
    persist = ctx.enter_context(tc.tile_pool(name="persist", bufs=1))

    # ---------------- the one input DMA ----------------
    blob = consts.tile([128, BLOB], F16)
    nc.sync.dma_start(out=blob, in_=blob_ext[:, :])

    qmat = blob[:, ds(QOFF, KT * IJ)].rearrange("p (k i) -> p k i", k=KT)
    mks = blob[:, ds(MOFF, KT * 128)].rearrange("p (k m) -> p k m", k=KT)
    w2_sb = blob[:, ds(W2OFF, W)]
    w1r = blob[:, ds(W1ROFF, NT * H)].rearrange("p (q h) -> p q h", q=NT)
    b1_sb = blob[:, ds(B1OFF, 2)].bitcast(F32)
    r_sb = blob[:, ds(ROFF, NT * 3 * 2)].bitcast(F32).rearrange(
        "p (q c) -> p q c", c=3)

    id128 = consts.tile([128, 128], F32)
    make_identity(nc, id128)

    # persistent feature-major tensors
    ht = persist.tile([128, NP], F16)       # hidden h^T
    radt = persist.tile([W, NP], F16)       # Rad^T (no b2)
    y4 = persist.tile([128, NP], F16)       # Ysh^T rows at bases 0/32/64/96
    rad9 = persist.tile([128, 4, NP], F16)  # Rad rows in k-tile layout
    ost = persist.tile([128, NT, IJ], F16)  # output staging (one DMA)

    # ---------------- point-major precomputation ----------------
    # Block point order: z = 64*p + q
    prep = ctx.enter_context(tc.tile_pool(name="prep", bufs=1))
    rsq = prep.tile([128, NT, 3], F32)
    nc.vector.tensor_mul(rsq, r_sb, r_sb)
    rad2 = prep.tile([128, NT], F32)
    nc.vector.tensor_reduce(rad2, rsq, axis=mybir.AxisListType.X, op=OP.add)
    radii = prep.tile([128, NT], F32)
    nc.scalar.activation(radii, rad2, AF.Sqrt)
    invr = prep.tile([128, NT], F32)
    nc.vector.reciprocal(invr, rad2)                    # 1/rad^2
    nc.vector.tensor_mul(invr, invr, radii)             # -> 1/rad

    d = prep.tile([128, NT, 3], F32)
    for c in range(3):
        nc.vector.tensor_mul(d[:, :, c], r_sb[:, :, c], invr)
    e = prep.tile([128, NT, 3], F32)
    nc.vector.tensor_scalar_mul(e, d, SQ15)
    g = prep.tile([128, NT, 3], F32)
    nc.vector.tensor_scalar_mul(g, e, 0.5)

    # Ysh point-major, padded to 32 y-slots so PE transposes land 32-aligned
    yw = prep.tile([128, NT, NYP], F32)
    nc.vector.memset(yw[:, :, NY:], 0.0)
    nc.vector.memset(yw[:, :, 0], 1.0)
    nc.vector.tensor_scalar_mul(yw[:, :, 1], d[:, :, 1], SQ3)
    nc.vector.tensor_scalar_mul(yw[:, :, 2], d[:, :, 2], SQ3)
    nc.vector.tensor_scalar_mul(yw[:, :, 3], d[:, :, 0], SQ3)
    nc.vector.tensor_mul(yw[:, :, 4], e[:, :, 0], d[:, :, 1])
    nc.vector.tensor_mul(yw[:, :, 5], e[:, :, 1], d[:, :, 2])
    t2 = prep.tile([128, NT], F32)
    nc.vector.tensor_mul(t2, d[:, :, 2], d[:, :, 2])
    nc.vector.tensor_scalar(yw[:, :, 6], t2, 1.5 * SQ5, -0.5 * SQ5,
                            op0=OP.mult, op1=OP.add)
    nc.vector.tensor_mul(yw[:, :, 7], e[:, :, 0], d[:, :, 2])
    su = prep.tile([128, NT], F32)
    sv = prep.tile([128, NT], F32)
    nc.vector.tensor_mul(su, g[:, :, 0], d[:, :, 0])
    nc.vector.tensor_mul(sv, g[:, :, 1], d[:, :, 1])
    nc.vector.tensor_sub(yw[:, :, 8], su, sv)

    # ---------------- feature-major precomputation ----------------
    with tc.tile_pool(name="ps_t", bufs=1, space="PSUM") as ps_t, \
         tc.tile_pool(name="ps_y", bufs=2, space="PSUM") as ps_y, \
         tc.tile_pool(name="ps_h", bufs=3, space="PSUM") as ps_h, \
         tc.tile_pool(name="ps_r", bufs=2, space="PSUM") as ps_r, \
         tc.tile_pool(name="rw", bufs=2) as p_rw:
        # dummy transpose so the PE engine observes the identity-build
        # semaphore before the real transposes (is_transpose matmuls only
        # support a single sync-wait command in codegen).
        dummy = ps_y.tile([128, 128], F32, tag="yt")
        nc.tensor.transpose(dummy, id128, id128)

        # radii: [128, 64] -T-> [64, 128] -> SBUF fp16
        rt_ps = ps_t.tile([NT, 128], F32, tag="rt")
        nc.tensor.transpose(rt_ps, radii, id128)
        rts = p_rw.tile([NT, 128], F16, tag="rts")
        nc.scalar.copy(rts, rt_ps)

        # hidden h^T[h, m=128q+p] = relu(W1[h]*radiiT[q, p] + b1[h]) via
        # K=64 selector matmuls: w1rep[q] = e_q outer W1.
        for q in range(NT):
            hp = ps_h.tile([128, 128], F32, tag="hp")
            nc.tensor.matmul(out=hp, lhsT=w1r[0:NT, q, :], rhs=rts,
                             start=True, stop=True)
            nc.scalar.activation(ht[:, ts(q, 128)], hp, AF.Relu,
                                 bias=b1_sb, scale=1.0)

        # Rad^T[w, z] = sum_h W2[h, w] h^T[h, z]
        for c in range(NCH):
            rp = ps_r.tile([W, 512], F32, tag="rp")
            nc.tensor.matmul(out=rp, lhsT=w2_sb, rhs=ht[:, ts(c, 512)],
                             start=True, stop=True)
            nc.scalar.copy(radt[:, ts(c, 512)], rp)

        # Ysh^T rows: per chunk transpose [128, 4*32] -> [4*32, 128];
        # rows 32t..32t+8 hold Y[z, y] for z-tile 4c+t.
        for c in range(NCH):
            yt_ps = ps_y.tile([128, 128], F32, tag="yt")
            nc.tensor.transpose(yt_ps, yw[:, ds(4 * c, 4), :], id128)
            for t in range(4):
                nc.vector.tensor_copy(y4[0:NY, ds(512 * c + 128 * t, 128)],
                                      yt_ps[ds(32 * t, NY), :])

    # replicate Ysh^T rows to partition bases 32/64/96 (row-tiled Ybc MMs)
    for gb in (32, 64, 96):
        nc.vector.tensor_copy(y4[gb:gb + NY, :], y4[0:NY, :])

    # rad9: Rad rows in k-tile partition layout (+1.0 for the B-block rows)
    for s, p0, w0, L in RAD9_COPIES:
        nc.vector.tensor_copy(rad9[p0:p0 + L, s, :], radt[w0:w0 + L, :])
    nc.vector.memset(rad9[96:128, 3, :], 1.0)

    # ---------------- main loop ----------------
    p_ft = ctx.enter_context(tc.tile_pool(name="ft", bufs=2))
    p_ybs = ctx.enter_context(tc.tile_pool(name="ybs", bufs=3))
    p_ybc = ctx.enter_context(tc.tile_pool(name="ybc", bufs=4, space="PSUM"))
    p_ops = ctx.enter_context(tc.tile_pool(name="ops", bufs=3, space="PSUM"))

    for c in range(NCH):
        ft = p_ft.tile([128, KT, 512], F16, tag="ft")
        for kt in range(KT):
            gb = 32 * (kt % 4)
            ybc = p_ybc.tile([128, 512], F32, tag="ybc")
            nc.tensor.matmul(out=ybc, lhsT=mks[gb:gb + NY, kt, :],
                             rhs=y4[gb:gb + NY, ts(c, 512)],
                             start=True, stop=True, tile_position=(gb, 0))
            rad_s = rad9[:, RAD9_SLOT[kt], ts(c, 512)]
            if kt in ACT_KTS:
                ybs = p_ybs.tile([128, 512], F16, tag="ybs")
                nc.scalar.copy(ybs, ybc)
                nc.vector.tensor_mul(ft[:, kt, :], ybs, rad_s)
            else:
                nc.vector.tensor_mul(ft[:, kt, :], ybc, rad_s)

        for j in range(4):
            t = 4 * c + j
            op = p_ops.tile([128, IJ], F32, tag="op")
            for kt in range(KT):
                nc.tensor.matmul(out=op, lhsT=ft[:, kt, ts(j, 128)],
                                 rhs=qmat[:, kt, :],
                                 start=(kt == 0), stop=(kt == KT - 1))
            nc.scalar.copy(ost[:, t, :], op)

    # ---------------- the one output DMA ----------------
    # z = 64*p + t: partition p's 64 rows are contiguous in out_ext
    nc.sync.dma_start(out=out_ext.rearrange("(p t) i -> p t i", t=NT), in_=ost)


def build_nc(repeat: int = 1) -> bass.Bass:
    nc = bacc.Bacc()
    blob_ext = nc.declare_dram_parameter("blob", [128, BLOB], F16,
                                         isOutput=False)
    out_ext = nc.declare_dram_parameter("out", [NP, IJ], F16, isOutput=True)
    with tile.TileContext(nc) as tc:
        for _ in range(repeat):
            _emit(tc, blob_ext, out_ext)
    nc.compile()
    return nc


def pack_weights(Q, b2):
    """Qstack = [Qmat; B; 0] in fp16, laid out [128, KT, IJ] (K-tile major)."""
    Q = np.asarray(Q, np.float32)
    b2 = np.asarray(b2, np.float32)
    qmat = Q.transpose(2, 3, 0, 1).reshape(KF, IJ)          # [(y,w), (i,j)]
    bmat = np.tensordot(b2, Q, axes=([0], [3]))             # [16,16,9]
    bmat = bmat.transpose(2, 0, 1).reshape(NY, IJ)
    qstack = np.zeros((128 * KT, IJ), np.float16)
    qstack[:KF] = qmat.astype(np.float16)
    qstack[KF:KV] = bmat.astype(np.float16)
    return np.ascontiguousarray(
        qstack.reshape(KT, 128, IJ).transpose(1, 0, 2))


def pack_mks():
    """Ybc selector: mks[32g + y, kt, p] = 1 iff feature row 128kt+p uses
    Ysh component y (replicated at 4 partition bases for PE row-tiling)."""
    mks = np.zeros((128, KT, 128), np.float16)
    for kt in range(KT):
        for p in range(128):
            k = 128 * kt + p
            if k < KF:
                y = k // W
            elif k < KV:
                y = k - KF
            else:
                continue
            for gb in range(4):
                mks[32 * gb + y, kt, p] = 1.0
    return np.ascontiguousarray(mks)


def pack_blob_base(Q, W1, b1, W2, b2):
    """Everything except r: one [128, BLOB] fp16 array."""
    blob = np.zeros((128, BLOB), np.float16)
    blob[:, QOFF:QOFF + KT * IJ] = pack_weights(Q, b2).reshape(128, -1)
    blob[:, MOFF:MOFF + KT * 128] = pack_mks().reshape(128, -1)
    blob[:, W2OFF:W2OFF + W] = np.asarray(W2, np.float32).astype(np.float16)
    w1rep = np.zeros((128, NT, H), np.float16)
    w1f = np.asarray(W1, np.float32).reshape(H).astype(np.float16)
    for q in range(NT):
        w1rep[q, q, :] = w1f
    blob[:, W1ROFF:W1ROFF + NT * H] = w1rep.reshape(128, -1)
    b1f = np.asarray(b1, np.float32).reshape(H, 1).astype(np.float32)
    blob[:, B1OFF:B1OFF + 2] = b1f.view(np.float16)
    return blob


def fill_blob_r(blob_base, r_shard):
    """Per-core blob: base + the r shard (f32 bits, block layout z=64p+q)."""
    blob = blob_base.copy()
    rview = np.ascontiguousarray(r_shard.astype(np.float32)).reshape(128, -1)
    blob[:, ROFF:ROFF + NT * 3 * 2] = rview.view(np.float16)
    return blob


_NC_CACHE = None


def _get_nc():
    global _NC_CACHE
    if _NC_CACHE is None:
        _NC_CACHE = build_nc()
    return _NC_CACHE


def kernel(r, Q, W1, b1, W2, b2, K0):
    r = np.ascontiguousarray(np.asarray(r, dtype=np.float32))
    base = pack_blob_base(Q, W1, b1, W2, b2)
    in_maps = [{"blob": fill_blob_r(base, r[i * NP:(i + 1) * NP])}
               for i in range(N_CORES)]
    res = run_bass_kernel_spmd(_get_nc(), in_maps, list(range(N_CORES)))
    out = np.concatenate([res.results[i]["out"] for i in range(N_CORES)], 0)
    out = out.reshape(N_TOTAL, 16, 16).astype(np.float32)
    # exact reference semantics for |r| == 0 points (K0 fallback)
    zero = ~(np.linalg.norm(r, axis=1) > 0.0)
    if zero.any():
        out[zero] = np.asarray(K0, np.float32)[None]
    return out


# revision 12
# speedup vs baseline: 171.3057x; 171.3057x over previous
"""Trainium2 Bass kernel for the e3nn-style point kernel:

    out[z, i, j] = sum_{y,w} Q[i,j,y,w] * Ysh[z,y] * Rad[z,w]      (+ K0 fallback
                                                                     for |r|==0)
    Ysh = real spherical harmonics l=0,1,2 of d = r/|r|  (component norm)
    Rad = relu(|r| * W1 + b1) @ W2 + b2

This execution environment charges a large fixed cost per *static*
instruction (~40us) and per DMA instruction (~1ms); dynamic re-execution of
the same instructions in a hardware loop is essentially free, and the 8
SPMD cores pay their static costs in parallel.  The kernel is therefore
shaped to minimize STATIC instruction count:

  - 3 DMA instructions: one host-packed blob in (constants + r shard), one
    SBUF->SBUF radii-row flatten, one output out.
  - All PE matmuls in fp32: fp32 matmuls self-load their stationary operand
    (no separate InstLdweights instruction).
  - One tc.For_i hardware loop runs the entire per-512-point-chunk pipeline
    (hidden MLP -> Rad^T -> k-tile Rad replica matmuls -> Ysh transpose ->
    Y broadcast matmuls -> Khatri-Rao multiply -> main GEMM -> output copy)
    with fixed-address tiles, so its ~45 instructions are emitted once.
  - The `repeat` dimension (the harness measures a NEFF that runs the
    computation REPEAT times back-to-back on device) is an outer hardware
    loop around the whole body, so x1 and xR NEFFs are statically identical
    and the wall-clock slope isolates the true marginal execution time.
  - Main GEMM computes out^T (ij on partitions, z streaming, N=512); the
    host undoes the transpose.
  - PE stationary operands must have static offsets, so the Ysh slice is
    staged into a fixed tile by a DVE copy before its transpose.
  Point order inside a core is block-permuted (z = 64*p + q); the host
  gather undoes it.  Khatri-Rao feature row k = 96*y + w matches the
  host-packed Qstack = [Qmat; B; 0]; Rad row replicas and Y broadcast rows
  come from K<=96 selector matmuls (host-packed 0/1 matrices).
"""

import math
from contextlib import ExitStack

import numpy as np

import concourse.bass as bass
import concourse.mybir as mybir
import concourse.tile as tile
from concourse import bacc
from concourse._compat import with_exitstack
from concourse.bass import ds, ts
from concourse.bass_utils import run_bass_kernel_spmd
from concourse.masks import make_identity

F32 = mybir.dt.float32
F16 = mybir.dt.float16
AF = mybir.ActivationFunctionType
OP = mybir.AluOpType

N_TOTAL = 65536
N_CORES = 8
NP = N_TOTAL // N_CORES          # 8192 points per core
NT = NP // 128                   # 64 radii-transpose rows (z = 64p + q)
NCH = NP // 512                  # 16 chunks of 512 points
H = 128                          # MLP hidden
W = 96                           # MLP out / radial channels
NY = 9                           # spherical harmonics
NYP = 32                         # padded y-slots (transpose alignment)
IJ = 256                         # 16*16 outputs
KF = 9 * W                       # 864 true feature rows
KV = KF + NY                     # 873 rows incl. Ysh block (for the B term)
KT = 7                           # K-tiles of 128 (896 rows, 873 valid)

SQ3 = math.sqrt(3.0)
SQ5 = math.sqrt(5.0)
SQ15 = math.sqrt(15.0)

# f32-element offsets of the per-partition regions inside the input blob
QOFF = 0                         # qstack   [7, 256]
MOFF = QOFF + KT * IJ            # mks      [7, 128]
POFF = MOFF + KT * 128           # prad     [3, 128] Rad-replica selectors
W2OFF = POFF + 3 * 128           # w2       [96]
W1OFF = W2OFF + W                # w1       [128] (partition 0 row)
B1OFF = W1OFF + H                # b1       [1]
ROFF = B1OFF + 1                 # r        [64, 3]
BLOB = ROFF + NT * 3             # total f32 elems per partition


@with_exitstack
def _emit(ctx: ExitStack, tc: tile.TileContext, blob_ext, out_ext,
          repeat: int):
    nc = tc.nc

    consts = ctx.enter_context(tc.tile_pool(name="consts", bufs=1))
    prep = ctx.enter_context(tc.tile_pool(name="prep", bufs=1))
    work = ctx.enter_context(tc.tile_pool(name="work", bufs=1))
    psum = ctx.enter_context(tc.tile_pool(name="psum", bufs=1, space="PSUM"))

    with tc.For_i(0, repeat, 1):
        # ---------------- the one input DMA ----------------
        blob = consts.tile([128, BLOB], F32, tag="blob")
        nc.sync.dma_start(out=blob, in_=blob_ext[:, :])

        qmat = blob[:, ds(QOFF, KT * IJ)].rearrange("p (k i) -> p k i", k=KT)
        mks = blob[:, ds(MOFF, KT * 128)].rearrange("p (k m) -> p k m", k=KT)
        prad = blob[:, ds(POFF, 3 * 128)].rearrange("p (s m) -> p s m", s=3)
        w2_sb = blob[:, ds(W2OFF, W)]
        w1_sb = blob[0:1, ds(W1OFF, H)]
        b1_sb = blob[:, ds(B1OFF, 1)]
        r_sb = blob[:, ds(ROFF, NT * 3)].rearrange("p (q c) -> p q c", c=3)

        id128 = consts.tile([128, 128], F32, tag="id")
        make_identity(nc, id128)

        ostt = consts.tile([128, 2, NP], F16, tag="ostt")

        # ---------------- point-major precomputation (whole shard) -------
        rsq = prep.tile([128, NT, 3], F32, tag="rsq")
        nc.vector.tensor_mul(rsq, r_sb, r_sb)
        rad2 = prep.tile([128, NT], F32, tag="rad2")
        nc.vector.tensor_reduce(rad2, rsq, axis=mybir.AxisListType.X,
                                op=OP.add)
        radii = prep.tile([128, NT], F32, tag="radii")
        nc.scalar.activation(radii, rad2, AF.Sqrt)
        invr = prep.tile([128, NT], F32, tag="invr")
        nc.vector.reciprocal(invr, rad2)                    # 1/rad^2
        nc.vector.tensor_mul(invr, invr, radii)             # -> 1/rad

        d = prep.tile([128, NT, 3], F32, tag="d")
        for c in range(3):
            nc.vector.tensor_mul(d[:, :, c], r_sb[:, :, c], invr)

        # Ysh point-major, padded to 32 y-slots (transpose alignment)
        yw = prep.tile([128, NT, NYP], F32, tag="yw")
        nc.vector.memset(yw[:, :, NY:], 0.0)
        nc.vector.memset(yw[:, :, 0], 1.0)
        nc.vector.tensor_scalar_mul(yw[:, :, 1], d[:, :, 1], SQ3)
        nc.vector.tensor_scalar_mul(yw[:, :, 2], d[:, :, 2], SQ3)
        nc.vector.tensor_scalar_mul(yw[:, :, 3], d[:, :, 0], SQ3)
        nc.vector.scalar_tensor_tensor(yw[:, :, 4], d[:, :, 0], SQ15,
                                       d[:, :, 1], op0=OP.mult, op1=OP.mult)
        nc.vector.scalar_tensor_tensor(yw[:, :, 5], d[:, :, 1], SQ15,
                                       d[:, :, 2], op0=OP.mult, op1=OP.mult)
        t2 = prep.tile([128, NT], F32, tag="t2")
        nc.vector.tensor_mul(t2, d[:, :, 2], d[:, :, 2])
        nc.vector.tensor_scalar(yw[:, :, 6], t2, 1.5 * SQ5, -0.5 * SQ5,
                                op0=OP.mult, op1=OP.add)
        nc.vector.scalar_tensor_tensor(yw[:, :, 7], d[:, :, 0], SQ15,
                                       d[:, :, 2], op0=OP.mult, op1=OP.mult)
        su = prep.tile([128, NT], F32, tag="su")
        sv = prep.tile([128, NT], F32, tag="sv")
        nc.vector.scalar_tensor_tensor(su, d[:, :, 0], 0.5 * SQ15,
                                       d[:, :, 0], op0=OP.mult, op1=OP.mult)
        nc.vector.scalar_tensor_tensor(sv, d[:, :, 1], 0.5 * SQ15,
                                       d[:, :, 1], op0=OP.mult, op1=OP.mult)
        nc.vector.tensor_sub(yw[:, :, 8], su, sv)

        # radii row: [128, 64] -T-> [64, 128] -> SBUF -> flatten to [1, NP].
        # The dummy transpose makes the PE observe the identity-build
        # semaphore first (is_transpose matmuls only support one sync-wait).
        rts = prep.tile([NT, 128], F32, tag="rts")
        rrow = prep.tile([1, NP], F32, tag="rrow")
        dummy = psum.tile([128, 128], F32, tag="small")
        nc.tensor.transpose(dummy, id128, id128)
        rt_ps = psum.tile([NT, 128], F32, tag="small")
        nc.tensor.transpose(rt_ps, radii, id128)
        nc.vector.tensor_copy(rts, rt_ps)
        nc.sync.dma_start(out=rrow[0:1, :], in_=rts[:, :])

        # ---------------- the per-chunk hardware loop ----------------
        with tc.For_i(0, NCH, 1) as i:
            # hidden h^T = relu(W1 r + b1);  Rad^T = W2^T h^T  (this chunk)
            hp = psum.tile([128, 512], F32, tag="small")
            nc.tensor.matmul(out=hp, lhsT=w1_sb, rhs=rrow[0:1, ts(i, 512)],
                             start=True, stop=True)
            htc = work.tile([128, 512], F32, tag="htc")
            nc.vector.tensor_scalar(htc, hp, b1_sb, 0.0,
                                    op0=OP.add, op1=OP.max)
            rp = psum.tile([W, 512], F32, tag="small")
            nc.tensor.matmul(out=rp, lhsT=w2_sb, rhs=htc,
                             start=True, stop=True)
            radc = work.tile([W, 512], F32, tag="radc")
            nc.vector.tensor_copy(radc, rp)

            # Rad rows in k-tile partition layout via selector matmuls
            # (DVE may read only one PSUM operand per op, so stage to SBUF)
            rad9ps = psum.tile([128, 3, 512], F32, tag="big")
            for s in range(3):
                nc.tensor.matmul(out=rad9ps[:, s, :], lhsT=prad[0:W, s, :],
                                 rhs=radc, start=True, stop=True)
            rad9 = work.tile([128, 3, 512], F32, tag="rad9")
            nc.vector.tensor_copy(rad9, rad9ps)

            # Ysh^T rows for this chunk: [128, 4*32] -T-> rows 32t+y -> y4c
            ywc = work.tile([128, 4, NYP], F32, tag="ywc")
            nc.vector.tensor_copy(ywc, yw[:, ds(4 * i, 4), :])
            yt_ps = psum.tile([128, 128], F32, tag="small")
            nc.tensor.transpose(yt_ps, ywc, id128)
            y4c = work.tile([NY, 512], F32, tag="y4c")
            for t in range(4):
                nc.vector.tensor_copy(y4c[0:NY, ds(128 * t, 128)],
                                      yt_ps[ds(32 * t, NY), :])

            # Khatri-Rao features F^T = Ybc * rad9 (k-tiles 0..2 then 3..5
            # reuse the Ybc banks; k-tile 6 = Y8*Rad rows + B-block rows)
            ft = work.tile([128, KT, 512], F32, tag="ft")
            ybc3 = psum.tile([128, 3, 512], F32, tag="ybc3")
            ybc6 = psum.tile([128, 512], F32, tag="ybc6")
            for h in range(2):
                for s in range(3):
                    nc.tensor.matmul(out=ybc3[:, s, :],
                                     lhsT=mks[0:NY, 3 * h + s, :],
                                     rhs=y4c, start=True, stop=True)
                nc.vector.tensor_mul(ft[:, ds(3 * h, 3), :], ybc3, rad9)
            nc.tensor.matmul(out=ybc6, lhsT=mks[0:NY, 6, :], rhs=y4c,
                             start=True, stop=True)
            nc.vector.tensor_mul(ft[0:W, 6, :], ybc6[0:W, :], radc)
            nc.vector.tensor_copy(ft[W:128, 6, :], ybc6[W:128, :])

            # main GEMM: out^T[ij, z] = sum_k Qstack[k, ij] F^T[k, z]
            op2 = psum.tile([128, 2, 512], F32, tag="big")
            for a in range(2):
                for kt in range(KT):
                    nc.tensor.matmul(out=op2[:, a, :],
                                     lhsT=qmat[:, kt, ds(128 * a, 128)],
                                     rhs=ft[:, kt, :],
                                     start=(kt == 0), stop=(kt == KT - 1))
            nc.vector.tensor_copy(ostt[:, :, ts(i, 512)], op2)

        # ---------------- the one output DMA ----------------
        # ostt[p_ij, a, m] = out[z(m), ij=128a+p_ij]
        nc.sync.dma_start(out=out_ext.rearrange("(a p) z -> p a z", p=128),
                          in_=ostt)


def build_nc(repeat: int = 1) -> bass.Bass:
    nc = bacc.Bacc()
    blob_ext = nc.declare_dram_parameter("blob", [128, BLOB], F32,
                                         isOutput=False)
    out_ext = nc.declare_dram_parameter("out", [IJ, NP], F16, isOutput=True)
    with tile.TileContext(nc) as tc:
        _emit(tc, blob_ext, out_ext, repeat)
    nc.compile()
    return nc


def pack_weights(Q, b2):
    """Qstack = [Qmat; B; 0] in fp32, laid out [128, KT, IJ] (K-tile major)."""
    Q = np.asarray(Q, np.float32)
    b2 = np.asarray(b2, np.float32)
    qmat = Q.transpose(2, 3, 0, 1).reshape(KF, IJ)          # [(y,w), (i,j)]
    bmat = np.tensordot(b2, Q, axes=([0], [3]))             # [16,16,9]
    bmat = bmat.transpose(2, 0, 1).reshape(NY, IJ)
    qstack = np.zeros((128 * KT, IJ), np.float32)
    qstack[:KF] = qmat
    qstack[KF:KV] = bmat
    return np.ascontiguousarray(
        qstack.reshape(KT, 128, IJ).transpose(1, 0, 2))


def pack_mks():
    """Ybc selector: mks[y, kt, p] = 1 iff feature row 128kt+p uses Ysh
    component y (rows 9..127 unused)."""
    mks = np.zeros((128, KT, 128), np.float32)
    for kt in range(KT):
        for p in range(128):
            k = 128 * kt + p
            if k < KF:
                y = k // W
            elif k < KV:
                y = k - KF
            else:
                continue
            mks[y, kt, p] = 1.0
    return np.ascontiguousarray(mks)


def pack_prad():
    """Rad-replica selectors: prad[w, s, p] = 1 iff feature row 128kt+p
    (kt in {s, s+3}) uses Rad channel w; k-tiles repeat with period 3."""
    prad = np.zeros((128, 3, 128), np.float32)
    for s in range(3):
        for p in range(128):
            k = 128 * s + p
            if k < KF:
                prad[k % W, s, p] = 1.0
    return np.ascontiguousarray(prad)


def pack_blob_base(Q, W1, b1, W2, b2):
    """Everything except r: one [128, BLOB] f32 array."""
    blob = np.zeros((128, BLOB), np.float32)
    blob[:, QOFF:QOFF + KT * IJ] = pack_weights(Q, b2).reshape(128, -1)
    blob[:, MOFF:MOFF + KT * 128] = pack_mks().reshape(128, -1)
    blob[:, POFF:POFF + 3 * 128] = pack_prad().reshape(128, -1)
    blob[:, W2OFF:W2OFF + W] = np.asarray(W2, np.float32)
    blob[0, W1OFF:W1OFF + H] = np.asarray(W1, np.float32).reshape(H)
    blob[:, B1OFF] = np.asarray(b1, np.float32).reshape(H)
    return blob


def fill_blob_r(blob_base, r_shard):
    """Per-core blob: base + the r shard (block layout z = 64p + q)."""
    blob = blob_base.copy()
    blob[:, ROFF:ROFF + NT * 3] = np.ascontiguousarray(
        r_shard.astype(np.float32)).reshape(128, NT * 3)
    return blob


def unpack_out(dev_out):
    """[IJ, NP] fp16 device layout -> [NP, 16, 16] f32.

    dev_out[(a, mm), m] = out[z, ij] with ij = 128a + mm and, for
    m = 128q + p, z = 64p + q."""
    arr = np.asarray(dev_out, np.float32).reshape(2, 128, NT, 128)
    return arr.transpose(3, 2, 0, 1).reshape(NP, 16, 16)


_NC_CACHE = None


def _get_nc():
    global _NC_CACHE
    if _NC_CACHE is None:
        _NC_CACHE = build_nc()
    return _NC_CACHE


def kernel(r, Q, W1, b1, W2, b2, K0):
    r = np.ascontiguousarray(np.asarray(r, dtype=np.float32))
    base = pack_blob_base(Q, W1, b1, W2, b2)
    in_maps = [{"blob": fill_blob_r(base, r[i * NP:(i + 1) * NP])}
               for i in range(N_CORES)]
    res = run_bass_kernel_spmd(_get_nc(), in_maps, list(range(N_CORES)))
    out = np.concatenate([unpack_out(res.results[i]["out"])
                          for i in range(N_CORES)], 0)
    # exact reference semantics for |r| == 0 points (K0 fallback)
    zero = ~(np.linalg.norm(r, axis=1) > 0.0)
    if zero.any():
        out[zero] = np.asarray(K0, np.float32)[None]
    return out
